# revision 1
# baseline (speedup 1.0000x reference)
"""Fused multi-head self-attention (degenerate seq-len-1) + LayerNorm for TRN2.

Math: with sequence length 1, softmax over the single key is exactly 1.0, so
attention output == v.  The whole module collapses to

    out = LayerNorm((x @ W_v.T + b_v) @ W_proj.T + b_proj) * gamma + beta
        = LayerNorm(x @ C.T + bias) * gamma + beta

with C = W_proj @ W_v and bias = W_proj @ b_v + b_proj (both batch-independent,
folded on the host).  The device kernel is a single [1024,4096]x[4096,4096]
matmul per core (batch data-parallel over 8 cores) fused with LayerNorm.
"""

import os
import sys

import numpy as np

if "/opt/trn_rl_repo" not in sys.path:
    sys.path.insert(0, "/opt/trn_rl_repo")

import ml_dtypes

P = 128              # SBUF partitions
DIM = 4096
B = 8192
NCORES = 8
BL = B // NCORES     # batch rows per core
BT = BL // P         # b tiles per core
KO = DIM // P        # contraction tiles
JC = 256             # moving free-dim chunk (output cols per matmul)
NJC = DIM // JC
EPS = 1e-5

_BUILD_CACHE = {}


def _build(apply_bias: bool, apply_affine: bool):
    key = (apply_bias, apply_affine)
    if key in _BUILD_CACHE:
        return _BUILD_CACHE[key]

    import concourse.mybir as mybir
    import concourse.tile as tile
    from concourse import bacc

    bf16 = mybir.dt.bfloat16
    f16 = mybir.dt.float16
    f32 = mybir.dt.float32

    nc = bacc.Bacc("TRN2", target_bir_lowering=False, debug=False,
                   num_devices=NCORES)

    xt_d = nc.declare_dram_parameter("xt", [BT, P, KO, P], bf16, isOutput=False)
    ct_d = nc.declare_dram_parameter("ct", [NJC, P, KO, JC], bf16, isOutput=False)
    bias_d = nc.declare_dram_parameter("bias", [DIM], f32, isOutput=False)
    gamma_d = nc.declare_dram_parameter("gamma", [DIM], f32, isOutput=False)
    beta_d = nc.declare_dram_parameter("beta", [DIM], f32, isOutput=False)
    # fp16 output (upcast on host): halves the output traffic so the final
    # writes fit under the last compute window; 10 mantissa bits is plenty
    # for LayerNorm-scale values.
    out_d = nc.declare_dram_parameter("out", [BT, P, NJC, JC], f16, isOutput=True)

    with tile.TileContext(nc) as tc:
        with tc.tile_pool(name="xpool", bufs=1) as xpool, \
             tc.tile_pool(name="wpool", bufs=3) as wpool, \
             tc.tile_pool(name="ypool", bufs=1) as ypool, \
             tc.tile_pool(name="spool", bufs=1) as spool, \
             tc.tile_pool(name="opool", bufs=2) as opool, \
             tc.tile_pool(name="small", bufs=4) as small, \
             tc.tile_pool(name="ppool", bufs=4, space="PSUM") as ppool:

            # Prefetch C chunk 0, then x b-tile 0, then C chunk 1, then the
            # remaining x b-tiles — the PE's first accumulation group needs
            # only ct[0] + xt[bt0] (3 MiB), not the full 8 MiB of x.
            ct_tiles = {}
            xt_sb = xpool.tile([P, BT, KO, P], bf16)

            # PE warmup: independent matmuls on a memset scratch tile run
            # during the initial DMA head (no data deps), so the HAM clock
            # gate reaches 2.4 GHz before the real matmuls start.
            warm_sb = small.tile([P, 384], bf16)
            nc.gpsimd.memset(warm_sb, 0.0)
            warm_ps = ppool.tile([P, JC], f32, name="warm_ps", tag="ps")
            for _ in range(56):
                nc.tensor.matmul(warm_ps, lhsT=warm_sb[:, 0:P],
                                 rhs=warm_sb[:, P:P + JC],
                                 start=True, stop=True)

            # Split the first chunk's DMAs so the first accumulation group's
            # dependencies resolve after ~0.4 MiB instead of 3 MiB.
            ct_tiles[0] = wpool.tile([P, KO, JC], bf16, name="ct_sb", tag="ct")
            first_splits = [0, 4, 8, 16, 24]
            for a, b in zip(first_splits, first_splits[1:] + [KO]):
                nc.sync.dma_start(out=xt_sb[:, 0, a:b],
                                  in_=xt_d[0, :, a:b])
                nc.sync.dma_start(out=ct_tiles[0][:, a:b],
                                  in_=ct_d[0, :, a:b])
            ct_tiles[1] = wpool.tile([P, KO, JC], bf16, name="ct_sb", tag="ct")
            nc.sync.dma_start(out=ct_tiles[1], in_=ct_d[1])
            for bt in range(1, BT):
                nc.sync.dma_start(out=xt_sb[:, bt], in_=xt_d[bt])

            # y (pre-norm matmul result) stays resident in bf16.
            y_sb = ypool.tile([P, BT, NJC, JC], bf16)
            # Per-chunk bn_stats, aggregated per b-tile at the end.
            stats_sb = spool.tile([P, BT, NJC, 6], f32)

            eps_sb = small.tile([P, 1], f32)
            nc.vector.memset(eps_sb, EPS)

            bias_sb = None
            if apply_bias:
                bias_sb = spool.tile([P, NJC, JC], f32)
                nc.sync.dma_start(out=bias_sb,
                                  in_=bias_d.ap().to_broadcast([P, NJC, JC]))

            gamma_sb = beta_sb = None
            if apply_affine:
                gamma_sb = spool.tile([P, NJC, JC], f32)
                nc.sync.dma_start(out=gamma_sb,
                                  in_=gamma_d.ap().to_broadcast([P, NJC, JC]))
                beta_sb = spool.tile([P, NJC, JC], f32)
                nc.sync.dma_start(out=beta_sb,
                                  in_=beta_d.ap().to_broadcast([P, NJC, JC]))

            JL = NJC // 2  # two LayerNorm-apply chunks of 2048 columns

            def layernorm_apply(bt, last_ps=None):
                """Aggregate stats and write the normalized b-tile.

                The apply is (y - mu) * rstd, computed as two half-tiles:
                one on the Scalar engine (Identity(y*rstd + (-mu*rstd)) with
                per-partition scale/bias) and one on DVE (tensor_scalar), so
                each engine's per-b-tile work stays under the PE shadow of
                the remaining matmuls.  Out-DMAs issue from the idle GpSimd
                queue to keep Sync free.
                """
                mv = small.tile([P, 2], f32)
                nc.vector.bn_aggr(mv, stats_sb[:, bt, :, :])
                std = small.tile([P, 1], f32)
                nc.scalar.activation(std, mv[:, 1:2],
                                     mybir.ActivationFunctionType.Sqrt,
                                     bias=eps_sb)
                rstd = small.tile([P, 1], f32)
                nc.vector.reciprocal(rstd, std)
                nmr = small.tile([P, 1], f32)
                nc.vector.tensor_scalar(
                    nmr, mv[:, 0:1], scalar1=rstd, scalar2=-1.0,
                    op0=mybir.AluOpType.mult, op1=mybir.AluOpType.mult,
                )
                for i, j0 in enumerate(range(0, NJC, JL)):
                    o = opool.tile([P, JL, JC], f16)
                    if i == 0:
                        nc.scalar.activation(
                            o, y_sb[:, bt, j0:j0 + JL, :],
                            mybir.ActivationFunctionType.Identity,
                            bias=nmr, scale=rstd,
                        )
                    elif last_ps is not None:
                        # The final chunk never went through the PSUM->SBUF
                        # eviction; normalize it straight out of PSUM.
                        nc.vector.tensor_scalar(
                            o[:, :JL - 1, :], y_sb[:, bt, j0:j0 + JL - 1, :],
                            scalar1=mv[:, 0:1], scalar2=rstd,
                            op0=mybir.AluOpType.subtract,
                            op1=mybir.AluOpType.mult,
                        )
                        nc.vector.tensor_scalar(
                            o[:, JL - 1, :], last_ps,
                            scalar1=mv[:, 0:1], scalar2=rstd,
                            op0=mybir.AluOpType.subtract,
                            op1=mybir.AluOpType.mult,
                        )
                    else:
                        nc.vector.tensor_scalar(
                            o, y_sb[:, bt, j0:j0 + JL, :],
                            scalar1=mv[:, 0:1], scalar2=rstd,
                            op0=mybir.AluOpType.subtract,
                            op1=mybir.AluOpType.mult,
                        )
                    if apply_affine:
                        nc.vector.tensor_mul(o, o, gamma_sb[:, j0:j0 + JL, :])
                        nc.vector.tensor_add(o, o, beta_sb[:, j0:j0 + JL, :])
                    nc.gpsimd.dma_start(out=out_d[bt, :, j0:j0 + JL, :], in_=o)

            def matmul_group(ct_sb, jc, bt):
                ps = ppool.tile([P, JC], f32, name="ps", tag="ps")
                for ko in range(KO):
                    nc.tensor.matmul(
                        ps,
                        lhsT=xt_sb[:, bt, ko, :],
                        rhs=ct_sb[:, ko, :],
                        start=(ko == 0),
                        stop=(ko == KO - 1),
                    )
                final_chunk = jc == NJC - 1 and not apply_bias
                if apply_bias:
                    nc.vector.tensor_add(y_sb[:, bt, jc, :], ps,
                                         bias_sb[:, jc, :])
                    nc.vector.bn_stats(stats_sb[:, bt, jc, :],
                                       y_sb[:, bt, jc, :])
                else:
                    # DVE reads PSUM directly for the LayerNorm statistics;
                    # ACT evicts PSUM (cast to bf16) except for the final
                    # chunk, which the epilogue normalizes straight out of
                    # PSUM.
                    if not final_chunk:
                        nc.scalar.activation(
                            y_sb[:, bt, jc, :], ps,
                            mybir.ActivationFunctionType.Copy)
                    nc.vector.bn_stats(stats_sb[:, bt, jc, :], ps)
                if jc == NJC - 1:
                    # Interleave the LayerNorm epilogue with the remaining
                    # b-tiles' matmuls.
                    layernorm_apply(bt, last_ps=ps if final_chunk else None)

            for jc in range(NJC):
                if jc + 2 < NJC:
                    ct_tiles[jc + 2] = wpool.tile([P, KO, JC], bf16,
                                                  name="ct_sb", tag="ct")
                    nc.sync.dma_start(out=ct_tiles[jc + 2], in_=ct_d[jc + 2])
                ct_sb = ct_tiles.pop(jc)
                for bt in range(BT):
                    matmul_group(ct_sb, jc, bt)

    nc.compile()
    _BUILD_CACHE[key] = nc
    return nc


def kernel(x, W_qkv, b_qkv, W_proj, b_proj, gamma, beta):
    from concourse.bass_utils import run_bass_kernel_spmd

    x = np.asarray(x, dtype=np.float32)
    W_qkv = np.asarray(W_qkv, dtype=np.float32)
    b_qkv = np.asarray(b_qkv, dtype=np.float32)
    W_proj = np.asarray(W_proj, dtype=np.float32)
    b_proj = np.asarray(b_proj, dtype=np.float32)
    gamma = np.asarray(gamma, dtype=np.float32)
    beta = np.asarray(beta, dtype=np.float32)

    # Fold the two projections (q/k are dead: seq len 1 => attention == v).
    W_v = W_qkv[2 * DIM:3 * DIM, :]
    C = W_proj @ W_v                          # [j, k]
    bias_total = W_proj @ b_qkv[2 * DIM:] + b_proj

    # C^T tiled for streaming: ct[jc, p, ko, jl] = C[jc*JC+jl, ko*P+p]
    Ct = np.ascontiguousarray(
        C.T.reshape(KO, P, NJC, JC).transpose(2, 1, 0, 3)
    ).astype(ml_dtypes.bfloat16)

    apply_bias = bool(np.any(bias_total))
    apply_affine = not (np.all(gamma == 1.0) and np.all(beta == 0.0))

    nc = _build(apply_bias, apply_affine)

    in_maps = []
    for i in range(NCORES):
        xs = x[i * BL:(i + 1) * BL]           # [BL, DIM]
        # xt[bt, p, ko, b'] = xs[bt*P + b', ko*P + p]
        xt = np.ascontiguousarray(
            xs.T.reshape(KO, P, BT, P).transpose(2, 1, 0, 3)
        ).astype(ml_dtypes.bfloat16)
        in_maps.append({
            "xt": xt,
            "ct": Ct,
            "bias": bias_total,
            "gamma": gamma,
            "beta": beta,
        })

    trace = bool(int(os.environ.get("KERNEL_TRACE", "0")))
    res = run_bass_kernel_spmd(nc, in_maps, core_ids=list(range(NCORES)),
                               trace=trace)
    if trace:
        kernel.last_exec_time_ns = res.exec_time_ns
        kernel.last_results = res

    out = np.concatenate(
        [r["out"].reshape(BL, DIM).astype(np.float32) for r in res.results],
        axis=0,
    )
    return out

